# revision 73
# baseline (speedup 1.0000x reference)
"""Multi-head self-attention (B=2, S=2048, D=1024, H=16) on 8 TRN2 NeuronCores.

Sharding: batch*heads tensor-parallel. Each core owns 2 heads (both batches):
QKV projection for its heads (W_qkv output-dim sharded), full attention for
its 2x2 (batch, head) pairs, partial output projection (W_out input-dim
sharded). Host sums the 8 fp16 partials + bias.

Schedule: the attention core is paced by the ACT engine's exp throughput
(4 full 2048x2048 score matrices per core, ~1.15us per [128,1024] exp
call, ~147us total); everything else is interleaved into the PE's slack
around it:

  - 8 attention "runs" of (batch, 512-token q-chunk) x 16 k-tiles, BOTH
    heads per window. Per k-step: 2 scores matmuls (K=64, N=512) issued as
    a row-tiled pair - head 0 in PE rows 0-63, head 1 in rows 64-127, so
    the hardware overlaps them - into one [128,1024] fp32 psum pair-tile
    (h0|h1); ONE exp call over both -> fp16 probs; 2 AV matmuls (vaug fp16
    [128,65] with a trailing ones column yields output AND softmax
    denominator in psum rows 0-64).
  - PSUM: scores pair-tile double-buffered (4 banks) so exp runs
    back-to-back, AV accumulators 2 banks, 2 banks for overlay work.
  - An overlay FIFO of single-matmul closures fills the PE slack in each
    window: batch-0's remaining q projections and v-transposes during its
    own early runs, batch-1's full QKV projection + v-transposes during
    batch-0's runs, and the output projection of completed q-chunks one
    run later (so its cross-engine dependency chain clears first). <=2-3
    pops per k-step bounds how long the in-order PE queue can block the
    scores->exp chain; hard deadlines are enforced by tagged flushes.
  - Softmax: accumulators are evacuated UNnormalized into fp16 oT (frees
    the psum banks fastest); denominator row -> reciprocal_approx_fast
    (DVE, ~5x faster than reciprocal) -> fp16 -> GPSIMD partition-
    broadcast -> 2 in-place DVE multiplies normalize oT later, off every
    critical engine. The final run instead uses K=1 PE broadcast matmuls
    (PE idles in the tail) and multiplies straight out of psum, and moves
    its gathers/casts to the then-idle ACT engine.
  - Startup: deadline-ordered contiguous DMAs (x is host-pretransposed so
    each chunk is one dense transfer; w split in t-subtile pieces so the
    first projection chain starts as they land), exp table preloaded via a
    dummy activation, and a ~2us dummy-matmul burst warms the PE clock
    (HAM) before the first real matmul.
  - fp16 for all SBUF operands and the output partials; fp32 only in PSUM
    accumulation and the denominator/reciprocal path.
"""

import sys

for _p in ("/opt/trn_rl_repo", "/root/.axon_site/_ro/trn_rl_repo"):
    if _p not in sys.path:
        sys.path.insert(0, _p)

from contextlib import ExitStack

import numpy as np

import concourse.bacc as bacc
import concourse.bass as bass
import concourse.mybir as mybir
import concourse.tile as tile
from concourse.bass_utils import run_bass_kernel_spmd
from concourse.masks import make_identity

F32 = mybir.dt.float32
F32R = mybir.dt.float32r
F16 = mybir.dt.float16

B, S, D, H = 2, 2048, 1024, 16
HD = D // H  # 64
T = B * S  # 4096 tokens
SCALE = HD**-0.5
N_CORES = 8
HEADS_PER_CORE = H // N_CORES  # 2

EXP = mybir.ActivationFunctionType.Exp


def build_kernel() -> bacc.Bacc:
    nc = bacc.Bacc(target_bir_lowering=False)
    # xp: host-pretransposed x, fully contiguous per (batch, 512-chunk) so
    # each chunk DMA is one dense [128, 8, 512] transfer on both sides.
    xp = nc.dram_tensor("xp", [128, B, 4, 8, 512], F16, kind="ExternalInput")
    wqkvT = nc.dram_tensor("wqkvT", [D, 6 * HD], F16, kind="ExternalInput")
    woutT = nc.dram_tensor("woutT", [2 * HD, D], F16, kind="ExternalInput")
    out = nc.dram_tensor("out", [T, D], F16, kind="ExternalOutput")

    with tile.TileContext(nc) as tc, ExitStack() as ctx:
        const = ctx.enter_context(tc.tile_pool(name="const", bufs=1))
        sb = ctx.enter_context(tc.tile_pool(name="sb", bufs=1))
        ps = ctx.enter_context(tc.tile_pool(name="ps", bufs=1, space="PSUM"))

        # DMAs in deadline order, split across queues; the first projection
        # chain can start as soon as the t=0 pieces of w and x(b0,ch0) land
        # (subtile deps).
        w_sb = const.tile([128, 8, 6 * HD], F16)
        wq_r = wqkvT.rearrange("(t p) c -> p t c", p=128)
        x_sb = {}
        for b in range(B):
            x_sb[b] = sb.tile([128, 4, 8, 512], F16, tag="x", bufs=2, name=f"x{b}")

        def dma_w(tp):
            nc.sync.dma_start(
                out=w_sb[:, 2 * tp : 2 * tp + 2, :], in_=wq_r[:, 2 * tp : 2 * tp + 2, :]
            )

        def dma_x(b, ch, thalf=None):
            tsl = slice(None) if thalf is None else slice(4 * thalf, 4 * thalf + 4)
            nc.sync.dma_start(out=x_sb[b][:, ch, tsl], in_=xp[:, b, ch, tsl])

        dma_w(0)
        dma_x(0, 0, 0)
        dma_w(1)
        dma_x(0, 0, 1)
        dma_w(2)
        dma_w(3)
        for ch in range(1, 4):
            dma_x(0, ch)
        for ch in range(4):
            dma_x(1, ch)
        wo = const.tile([2 * HD, D], F16)
        nc.sync.dma_start(out=wo, in_=woutT[:, :])

        ident = const.tile([128, 128], F16)
        make_identity(nc, ident)
        ones64 = const.tile([1, 64], F16)
        nc.vector.memset(ones64, 1.0)

        # PE warmup during the startup DMA wait: ~3.5us of dummy matmuls
        # trips the HAM activity monitor to full clock (2.4 GHz) before the
        # first real projection matmul.
        warm = ps.tile([64, 64], F32, tag="ov", bufs=2, name="warm")
        for _ in range(30):
            nc.tensor.matmul(
                warm[:], ident[:, 0:64], ident[:, 0:64], start=True, stop=True
            )

        # Preload ACT's exp table set during the startup DMAs.
        dummy_in = const.tile([1, 8], F32)
        nc.vector.memset(dummy_in, 0.0)
        dummy_out = const.tile([1, 8], F16)
        nc.scalar.activation(dummy_out[:], dummy_in[:], EXP, scale=SCALE)

        # Persistent SBUF tiles.
        qT, kT, oT = {}, {}, {}
        for b in range(B):
            qT[b] = sb.tile([128, S], F16, tag="qk", bufs=4, name=f"qT{b}")
            kT[b] = sb.tile([128, S], F16, tag="qk", bufs=4, name=f"kT{b}")
            oT[b] = sb.tile([128, S], F16, tag="ot", bufs=2, name=f"oT{b}")
        vT = sb.tile([128, S], F16, tag="vt", bufs=1, name="vT")
        vaug = {}
        for b in range(B):
            for ti in range(16):
                va = sb.tile([128, 130], F16, tag="vaug", bufs=32, name=f"va{b}_{ti}")
                nc.gpsimd.memset(va[:, 64:65], 1.0)
                nc.gpsimd.memset(va[:, 129:130], 1.0)
                vaug[b, ti] = va

        rec16 = {}  # (b, qc) -> [1, 1024] f16 reciprocal denominators (h0|h1)

        # ---------------- emission helpers ----------------

        def proj_entries(b, g, ch):
            """8 chained matmuls projecting x chunk ch through W group g
            (0=q, 1=k, 2=v), then a DVE evacuation to the fp16 destination."""
            state = {}
            csl = slice(ch * 512, (ch + 1) * 512)

            def entry(t):
                def run():
                    if t == 0:
                        state["acc"] = ps.tile(
                            [128, 512], F32, tag="ov", bufs=2, name="pacc"
                        )
                    nc.tensor.matmul(
                        state["acc"][:],
                        w_sb[:, t, g * 128 : (g + 1) * 128],
                        x_sb[b][:, ch, t, :],
                        start=(t == 0),
                        stop=(t == 7),
                    )
                    if t == 7:
                        dst = (qT[b], kT[b], vT)[g]
                        nc.vector.tensor_copy(dst[:, csl], state["acc"][:])

                return run

            return [entry(t) for t in range(8)]

        def transpose_entry(b, ti):
            def run():
                tp = ps.tile([128, 128], F16, tag="ov", bufs=2, name="tp")
                nc.tensor.transpose(tp[:], vT[:, ti * 128 : (ti + 1) * 128], ident[:])
                va = vaug[b, ti]
                # one strided copy fills both head slices (cols 0-63 and
                # 65-128), skipping the constant ones column at 64
                nc.vector.tensor_copy(
                    va[:, 0:130].rearrange("p (a c) -> p a c", a=2)[:, :, 0:64],
                    tp[:].rearrange("p (a c) -> p a c", a=2),
                )

            return run

        def p3_entries(b, qc, tail=False):
            """Normalize + project + store the 512 tokens of (b, qc).
            Normalization: broadcast the reciprocal row across 64
            partitions (GPSIMD mid-run where PE is precious; a pair of K=1
            PE matmuls in the tail where PE idles and the GPSIMD+DVE chain
            would gate) + DVE multiplies into oT. Then per 128-token chunk:
            2 out-proj matmuls + evacuation + DMA. Tail evacuations
            alternate DVE/ScalarE so the 2-bank psum WAR doesn't serialize
            at DVE speed."""
            entries = []
            csl = slice(qc * 512, (qc + 1) * 512)

            def norm_entry():
                # per-head, per-token normalization of the (unnormalized
                # fp16) oT, in place. The reciprocal row is broadcast to
                # all 128 partitions so each head's multiply reads its
                # operands at matching base partitions (DVE requirement).
                if tail:
                    # PE is idle in the tail and the GPSIMD broadcast
                    # would gate; K=1 matmuls are quicker, and the multiply
                    # reads the psum directly (PSUM operands are exempt
                    # from the DVE matching-base-partition rule).
                    for h in range(2):
                        p0 = h * 64
                        bcp = ps.tile([64, 512], F32, tag="ov", bufs=2, name="bcp")
                        nc.tensor.matmul(
                            bcp[:],
                            ones64[:],
                            rec16[b, qc][:, h * 512 : (h + 1) * 512],
                            start=True,
                            stop=True,
                        )
                        nc.vector.tensor_mul(
                            oT[b][p0 : p0 + 64, csl],
                            oT[b][p0 : p0 + 64, csl],
                            bcp[:],
                        )
                    return
                bcast = sb.tile([128, 1024], F16, tag="bcast", bufs=2, name="bcast")
                nc.gpsimd.partition_broadcast(bcast[:], rec16[b, qc][:])
                for h in range(2):
                    p0 = h * 64
                    nc.vector.tensor_mul(
                        oT[b][p0 : p0 + 64, csl],
                        oT[b][p0 : p0 + 64, csl],
                        bcast[p0 : p0 + 64, h * 512 : (h + 1) * 512],
                    )

            entries.append((1 if tail else 0, norm_entry))
            for tc_i in range(4 * qc, 4 * qc + 4):
                tsl = slice(tc_i * 128, (tc_i + 1) * 128)
                st2 = {}

                def op_entry(nk, tc_i=tc_i, tsl=tsl, st2=st2):
                    def run():
                        if nk == 0:
                            st2["ob"] = sb.tile(
                                [128, D], F16, tag="ob", bufs=4, name="ob"
                            )
                        nsl = slice(nk * 512, (nk + 1) * 512)
                        op = ps.tile([128, 512], F32, tag="ov", bufs=2, name="op")
                        nc.tensor.matmul(
                            op[:], oT[b][:, tsl], wo[:, nsl], start=True, stop=True
                        )
                        if tail and nk == 1:
                            nc.scalar.copy(st2["ob"][:, nsl], op[:])
                        else:
                            nc.vector.tensor_copy(st2["ob"][:, nsl], op[:])
                        if nk == 1:
                            r0 = b * S + tc_i * 128
                            nc.sync.dma_start(
                                out=out[r0 : r0 + 128, :], in_=st2["ob"][:]
                            )

                    return run

                entries.append((1, op_entry(0)))
                entries.append((1, op_entry(1)))
            return entries

        overlay = []  # FIFO of (tag, pe_cost, closure)
        pending_p3 = []  # p3 entries held back one run so their cross-engine
        # dependency chain (recip -> bcast -> mult) clears before the
        # out-proj matmuls enter the in-order PE queue

        def pop_overlay(budget):
            while overlay and budget > 0:
                _, cost, fn = overlay.pop(0)
                fn()
                budget -= cost

        def flush_overlay(tags):
            """Pop every entry whose tag is in tags, preserving relative
            order (entries of one chain share a tag, so chains stay
            in-order)."""
            rest = []
            for tag, cost, fn in overlay:
                if tag in tags:
                    fn()
                else:
                    rest.append((tag, cost, fn))
            overlay[:] = rest

        # ---------------- P1(b0): serial (ACT idle anyway) ----------------
        for ch in range(4):
            for e in proj_entries(0, 1, ch):  # k, all chunks
                e()
        for e in proj_entries(0, 0, 0):  # q chunk 0 (first run's q)
            e()
        for ch in range(2):  # v chunks 0-1 + their transposes
            for e in proj_entries(0, 2, ch):
                e()
            for ti in range(4 * ch, 4 * ch + 4):
                transpose_entry(0, ti)()

        # Overlay queue. b0's v chunks 2-3 drain first (run 1 pops at
        # elevated budget; vaug[4ch+j] is needed at run 1's k-step 4ch+j),
        # then b0's remaining q chunks, then b1's projections. b1's v+T
        # stay queued across the b1 flush and drain during run 5 the same
        # way b0's did in run 1.
        for ch in range(2, 4):
            overlay += [("p1b0v", 1, e) for e in proj_entries(0, 2, ch)]
            overlay += [
                ("p1b0v", 2, transpose_entry(0, ti)) for ti in range(4 * ch, 4 * ch + 4)
            ]
        for ch in range(1, 4):
            overlay += [(f"p1b0q{ch}", 1, e) for e in proj_entries(0, 0, ch)]
        # b1's v+T ahead of k/q: the v chain feeds run 5's earliest k-steps
        # (deadline), while k/q are only needed at run 5's start and can
        # take the boundary flush.
        for ch in range(4):
            overlay += [("p1b1v", 1, e) for e in proj_entries(1, 2, ch)]
            overlay += [
                ("p1b1v", 2, transpose_entry(1, ti)) for ti in range(4 * ch, 4 * ch + 4)
            ]
        for ch in range(4):
            overlay += [("p1b1kq", 1, e) for e in proj_entries(1, 1, ch)]
        overlay += [("p1b1kq", 1, e) for e in proj_entries(1, 0, 0)]
        # b1's q chunks 1-3 are only needed at runs 6-8: keep them out of
        # the boundary flush and let them drain as run-5+ pops, with the
        # same per-chunk flush backstops that b0's q chunks use
        for ch in range(1, 4):
            overlay += [(f"p1b1q{ch}", 1, e) for e in proj_entries(1, 0, ch)]

        # ---------------- attention runs ----------------
        for b in range(B):
            if b == 1:
                # everything of b1's projection that didn't drain in-run
                flush_overlay(
                    ("p1b0v", "p1b0q1", "p1b0q2", "p1b0q3", "p1b1v", "p1b1kq")
                )
            for qc in range(4):
                if b == 0 and qc > 0:
                    flush_overlay(
                        ("p1b0v",) + tuple(f"p1b0q{c}" for c in range(1, qc + 1))
                    )
                if b == 1 and qc > 0:
                    flush_overlay(tuple(f"p1b1q{c}" for c in range(1, qc + 1)))
                qsl = slice(qc * 512, (qc + 1) * 512)
                accs = [
                    ps.tile([65, 512], F32, tag="acc", bufs=2, name=f"av{b}{qc}{h}")
                    for h in range(2)
                ]
                prev = None
                for ki in range(16):
                    ksl = slice(ki * 128, (ki + 1) * 128)
                    sc = ps.tile([128, 1024], F32, tag="sc", bufs=2, name="sc")
                    for h in range(2):
                        p0 = h * 64
                        nc.tensor.matmul(
                            sc[:, h * 512 : (h + 1) * 512],
                            kT[b][p0 : p0 + 64, ksl],
                            qT[b][p0 : p0 + 64, qsl],
                            start=True,
                            stop=True,
                        )
                    pr = sb.tile([128, 1024], F16, tag="pr", bufs=4, name="pr")
                    nc.scalar.activation(pr[:], sc[:], EXP, scale=SCALE)
                    if prev is not None:
                        _av(nc, accs, vaug[b, prev[1]], prev[0], prev[1])
                    if ki == (2 if (b, qc) == (1, 3) else 8):
                        # the last run pops its predecessor's p3 early so
                        # those DVE evacuations clear before the tail's
                        # reciprocal chain needs the DVE
                        overlay.extend(pending_p3)
                        pending_p3 = []
                    if ki > 0:
                        # elevated budget where a v+T drain or the b1
                        # boundary has a deadline
                        pop_overlay(3 if (b, qc) in ((0, 0), (0, 3)) else 2)
                    prev = (pr, ki)
                _av(nc, accs, vaug[b, prev[1]], prev[0], prev[1])
                # epilogue: denominator rows first (they head the longest
                # chain: recip -> transpose -> scaled evacuation), then the
                # accumulators go STRAIGHT into unnormalized fp16 oT —
                # normalization happens after the output projection, where
                # tokens sit on partitions. The two oT copies also free the
                # accumulator psum banks for the next run's AV matmuls.
                tail = b == 1 and qc == 3
                den = sb.tile([1, 1024], F32, tag="den", bufs=2, name="den")
                for h in range(2):
                    # tail: ACT is idle after the last exp — run the
                    # denominator gathers there, parallel to the DVE's oT
                    # evacuations, shortening the tail's serial chain
                    (nc.scalar.copy if tail else nc.vector.tensor_copy)(
                        den[:, h * 512 : (h + 1) * 512], accs[h][64:65, :]
                    )
                for h in range(2):
                    p0 = h * 64
                    nc.vector.tensor_copy(oT[b][p0 : p0 + 64, qsl], accs[h][0:64, :])
                rec32 = sb.tile([1, 1024], F32, tag="rec32", bufs=2, name="rec32")
                nc.vector.reciprocal_approx_fast(rec32[:], den[:])
                r16 = sb.tile([1, 1024], F16, tag="rec16", bufs=8, name="rec16")
                (nc.scalar.copy if tail else nc.vector.tensor_copy)(r16[:], rec32[:])
                rec16[b, qc] = r16
                pending_p3 = [("p3", c, e) for (c, e) in p3_entries(b, qc, tail=tail)]

        overlay.extend(pending_p3)

        # ---------------- tail ----------------
        while overlay:
            overlay.pop(0)[2]()

    nc.finalize()
    return nc


def _av(nc, accs, va, pr, ki):
    """AV matmuls for one k-step: both heads accumulating into accs[h];
    lhsT = [v_h | 1] so row 64 accumulates the softmax denominator."""
    for h in range(2):
        nc.tensor.matmul(
            accs[h][:],
            va[:, h * 65 : (h + 1) * 65],
            pr[:, h * 512 : (h + 1) * 512],
            start=(ki == 0),
            stop=(ki == 15),
        )


_NC_CACHE = None
TRACE = False  # set True (e.g. from test.py) to capture an NTFF profile
LAST_RESULT = None  # BassKernelResults of the most recent run


def _get_nc():
    global _NC_CACHE
    if _NC_CACHE is None:
        _NC_CACHE = build_kernel()
    return _NC_CACHE


def kernel(x, W_qkv, W_out, b_out):
    x = np.asarray(x, dtype=np.float32)
    W_qkv = np.asarray(W_qkv, dtype=np.float32)
    W_out = np.asarray(W_out, dtype=np.float32)
    b_out = np.asarray(b_out, dtype=np.float32)

    xT = x.reshape(T, D).T.astype(np.float16)  # [D, T]
    # [p, b, ch, t, n] with d_model index d = t*128 + p, token = b*2048 + ch*512 + n
    xp = np.ascontiguousarray(
        xT.reshape(8, 128, B, 4, 512).transpose(1, 2, 3, 0, 4)
    )
    in_maps = []
    for c in range(N_CORES):
        h0 = c * HEADS_PER_CORE
        rows = slice(h0 * HD, (h0 + 2) * HD)  # this core's 128 head dims
        wq = W_qkv[0 * D :][rows]  # [128, D]
        wk = W_qkv[1 * D :][rows]
        wv = W_qkv[2 * D :][rows]
        wqkvT = np.ascontiguousarray(np.concatenate([wq, wk, wv], axis=0).T).astype(
            np.float16
        )
        woutT = np.ascontiguousarray(W_out[:, h0 * HD : (h0 + 2) * HD].T).astype(
            np.float16
        )
        in_maps.append({"xp": xp, "wqkvT": wqkvT, "woutT": woutT})

    nc = _get_nc()
    global LAST_RESULT
    res = run_bass_kernel_spmd(nc, in_maps, core_ids=list(range(N_CORES)), trace=TRACE)
    LAST_RESULT = res
    partial = np.zeros((T, D), dtype=np.float32)
    for c in range(N_CORES):
        partial += res.results[c]["out"].astype(np.float32)
    full = partial + b_out
    return full.astype(np.float32).reshape(B, S, D)
